# revision 24
# baseline (speedup 1.0000x reference)
# Self-contained Trainium2 Bass kernel for nn_MultiInputLSTMCell.
#
# Reference computation (all fp32):
#   pre   = h0 @ W_hh + bias + input_ @ W_ih          # (1, 3H)
#   i, o  = sigmoid(pre[:, :H]), sigmoid(pre[:, H:2H])
#   g     = tanh(pre[:, 2H:])
#   awi   = input_ @ aW_ih + a_bias                   # (1, H)
#   awh   = c_input @ aW_hh                           # (C, H)
#   alpha = sigmoid(awi + awh)                        # (C, H)
#   w     = exp([i; alpha]); w /= w.sum(0)            # (C+1, H)
#   c1    = (([g; c_input]) * w).sum(0)               # (1, H)
#   h1    = o * tanh(c1)
#
# Strategy: tensor-parallel over the hidden (output-column) dim across 8
# cores (HS = 256 columns each); all post-matmul work is local to a shard
# so there are no collectives.  Weights are host-cast to bf16 (~8.7 MB of
# HBM traffic per core; rel err ~2.5e-3 vs the 2e-2 gate).
#
# Schedule (the performance-critical part).  The kernel is memory-bound:
# one HWDGE ring streams weights at ~385 GB/s, so wall time ~= DMA span +
# whatever serial work trails the last byte.  Three measures keep the
# trailing work near zero:
#
#  1. DMA order [alpha weights | i,g gate columns | o gate columns]: the
#     whole softmax/c1/tanh chain depends only on the alpha and i/g
#     streams, so it completes while the o columns are still streaming.
#     After the last byte only sigma(o) (exp-form) and h1 = sigma(o) *
#     tanh(c1) remain (~1.5 us).
#  2. The (C+1)-row exp-normalize reduction is a K=64 ones-vector matmul
#     over the alpha rows only; the i/g row joins via DVE adds, not PE
#     matmuls, so no PE work exists after the o chunk stream and the PE
#     queue never stalls behind the activation chain.
#  3. The PE clock gate (activity monitor; cold = half clock) is warmed
#     by a block of back-to-back dummy matmuls issued right at kernel
#     start, while the first weight bytes are still in flight.  Without
#     this the whole matmul stream runs at half rate for its first ~15 us
#     and finishes ~8 us after the DMA does.
#
# All sigmoids except the very last are computed as exp + fast-reciprocal
# so the ACT engine loads the exp table once (pre-warmed at t=0) and
# never reloads mid-kernel; the final tanh(c1) is a sigmoid-family op
# after exps, which does not trigger a reload.  The g-gate weight columns
# are pre-scaled by 2 on the host (tanh(x) = 2*sigmoid(2x) - 1) so the
# same exp(-x) form serves the candidate gate.

import numpy as np

import concourse.bass as bass
import concourse.tile as tile
from concourse import bacc, mybir
from concourse.bass_utils import run_bass_kernel_spmd

NCORES = 8
H = 2048          # hidden size
IN = 2048         # input size
C = 64            # number of skip-word cell states
HS = H // NCORES  # hidden shard per core = 256
KG = IN + H       # gates contraction dim = 4096
KO_G = KG // 128  # 32 contraction chunks for the gates matmuls
KO_A = IN // 128  # 16 contraction chunks for the alpha matmuls
F32 = mybir.dt.float32
F32R = mybir.dt.float32r
BF16 = mybir.dt.bfloat16
FP8 = mybir.dt.float8e4

NPRIME = 10       # clock-gate priming matmuls (512 cols each)
# k-chunk sizes: big chunks while the PE is behind the stream, small once
# it catches up so per-chunk waits stay well under the ~3.4 us idle window
# that would re-throttle the PE clock.
IG_SIZES = [8, 8, 4, 4, 4, 4]  # k-chunk sizes for the [i|g] stream
O_SIZES = [12, 12, 6, 2]          # k-chunk sizes for the [o] stream (ramp-down)
ASCALE = 256.0    # fp8 pre-scale on the alpha weights (undone in the exp)

_nc_cache = None


def _build_nc():
    """Build the single-core Bass program (same program runs on all 8 cores)."""
    nc = bacc.Bacc(
        "TRN2",
        target_bir_lowering=False,
        debug=False,
        enable_asserts=False,
        name="multi_input_lstm_cell",
    )

    # DRAM I/O (per-core shards; shapes identical on every core).  Weights
    # are host-pre-tiled to [ki=128, ko, n] so each chunk DMA reads long
    # contiguous per-partition segments at full HBM efficiency.
    # xt[p, ko] = [h0; input_][ko*128 + p]
    xt = nc.dram_tensor("xt", [128, KO_G], BF16, kind="ExternalInput").ap()
    # ct[p, ko, c] = c_input[c, ko*128 + p]  (stationary for the alpha_hh matmul)
    ct = nc.dram_tensor("ct", [128, KO_A, C], FP8, kind="ExternalInput").ap()
    # walpha rows 0..15 = alpha_weight_ih shard, 16..31 = alpha_weight_hh shard
    wai = nc.dram_tensor("wai", [128, KO_A, HS], FP8, kind="ExternalInput").ap()
    wah = nc.dram_tensor("wah", [128, KO_A, HS], FP8, kind="ExternalInput").ap()
    # gates weights, columns [i_shard | 2*g_shard] and [o_shard]
    wgig = nc.dram_tensor("wgig", [128, KO_G, 2 * HS], FP8, kind="ExternalInput").ap()
    wgo = nc.dram_tensor("wgo", [128, KO_G, HS], BF16, kind="ExternalInput").ap()
    # bab[0, :] = [bias_i | 2*bias_g | bias_o | alpha_bias]
    bab = nc.dram_tensor("bab", [1, 4 * HS], F32, kind="ExternalInput").ap()
    cs = nc.dram_tensor("cs", [C, HS], F32R, kind="ExternalInput").ap()
    ones1 = nc.dram_tensor("ones1", [C, 1], F32R, kind="ExternalInput").ap()
    # hc[0, 0:256] = c1 shard, hc[0, 256:512] = h1 shard
    hc = nc.dram_tensor("hc", [1, 2 * HS], F32, kind="ExternalOutput").ap()

    with tile.TileContext(nc) as tc:
        _emit(tc, xt, ct, wai, wah, wgig, wgo, bab, cs, ones1, hc)

    nc.compile()
    return nc


def _emit(tc, xt, ct, wai, wah, wgig, wgo, bab, cs, ones1, hc):
    from contextlib import ExitStack

    nc = tc.nc
    EXP = mybir.ActivationFunctionType.Exp
    TANH = mybir.ActivationFunctionType.Tanh

    with ExitStack() as ctx:
        singles = ctx.enter_context(tc.tile_pool(name="singles", bufs=1))
        wig_pool = ctx.enter_context(tc.tile_pool(name="wig_pool", bufs=len(IG_SIZES)))
        wo_pool = ctx.enter_context(tc.tile_pool(name="wo_pool", bufs=len(O_SIZES)))
        psum = ctx.enter_context(tc.tile_pool(name="psum", bufs=1, space="PSUM"))

        # ---- tiny loads on the scalar (ACT) HWDGE ring; the big weight
        # stream owns the sync (SP) ring exclusively.
        bab_t = singles.tile([1, 4 * HS], F32, tag="bab")
        nc.scalar.dma_start(out=bab_t[:], in_=bab)

        # emw: [exp-weights | merge*exp-weights], rows 0..63 = alpha rows,
        # row 64 = the i/g gate row.  cs lands in the merge half early.
        emw = singles.tile([C + 1, 2 * HS], F32R, tag="emw")
        nc.scalar.dma_start(out=emw[0:C, HS : 2 * HS], in_=cs)

        ones_r = singles.tile([C, 1], F32R, tag="ones_r")
        nc.scalar.dma_start(out=ones_r[:], in_=ones1)


        ones_b = singles.tile([1, C], F32, tag="ones_b")
        nc.vector.memset(ones_b[:], 1.0)

        # Pre-warm the ACT exp table while everything else is idle.
        warm_t = singles.tile([1, 1], F32, tag="warm")
        nc.vector.memset(warm_t[:], 0.0)
        nc.scalar.activation(out=warm_t[:], in_=warm_t[:], func=EXP)

        # Priming fodder for the PE clock gate (contents irrelevant).
        prime_t = singles.tile([128, 512], BF16, tag="prime")
        nc.gpsimd.memset(prime_t[:], 0.0)

        # ---- sync-ring weight stream ----------------------------------
        # Order: xt, two big ig chunks (the ring ramps to full rate on big
        # bf16 transfers), the small fp8 alpha stream, remaining ig chunks,
        # o chunks, pad.  The PE consumes in the same order; the alpha
        # matmul block sits after two ig chunks so the PE reaches it just
        # as the alpha weights land (no multi-us stall, which would
        # re-throttle the PE clock).
        xt_t = singles.tile([128, KO_G], BF16, tag="xt")
        nc.sync.dma_start(out=xt_t[:], in_=xt)
        wig_ts = []
        kk = 0
        for ci, sz in enumerate(IG_SIZES):
            if ci == 2:
                ct_t = singles.tile([128, KO_A, C], FP8, tag="ct")
                nc.sync.dma_start(out=ct_t[:], in_=ct)
                wai_t = singles.tile([128, KO_A, HS], FP8, tag="wai")
                nc.sync.dma_start(out=wai_t[:], in_=wai)
                wah_t = singles.tile([128, KO_A, HS], FP8, tag="wah")
                nc.sync.dma_start(out=wah_t[:], in_=wah)
            wig_t = wig_pool.tile([128, max(IG_SIZES), 2 * HS], FP8, tag="wig")
            nc.sync.dma_start(out=wig_t[:, 0:sz, :], in_=wgig[:, kk : kk + sz, :])
            wig_ts.append((kk, sz, wig_t))
            kk += sz

        # ---- PSUM tiles ------------------------------------------------
        pdum = psum.tile([1, 512], F32, tag="pdum")     # priming scratch
        pg_ig = psum.tile([1, 2 * HS], F32, tag="pgig")  # [pre_i | 2*pre_g]
        pg_o = psum.tile([1, HS], F32, tag="pgo")        # pre_o
        pwi = psum.tile([1, HS], F32, tag="pwi")         # alpha_wi row
        pal = psum.tile([C, HS], F32, tag="pal")         # alpha pre-activation
        ps01 = psum.tile([1, 2 * HS], F32, tag="ps01")   # [sum ew | sum mg] over alpha rows

        # ---- PE: prime the clock gate with dense dummy matmuls ---------
        for _ in range(NPRIME):
            nc.tensor.matmul(pdum[:], lhsT=prime_t[:, 0:1], rhs=prime_t[:],
                             start=True, stop=True)

        # gate biases via K=1 rank-1 matmuls (open both gate PSUM groups)
        nc.tensor.matmul(pg_ig[:], lhsT=ones_b[0:1, 0:1], rhs=bab_t[:, 0 : 2 * HS],
                         start=True, stop=False)
        nc.tensor.matmul(pg_o[:], lhsT=ones_b[0:1, 0:1], rhs=bab_t[:, 2 * HS : 3 * HS],
                         start=True, stop=False)

        def ig_chunk(ci):
            kk, sz, wig_t = wig_ts[ci]
            for km in range(sz):
                nc.tensor.matmul(pg_ig[:], lhsT=xt_t[:, kk + km : kk + km + 1],
                                 rhs=wig_t[:, km, :],
                                 start=False, stop=(kk + km == KO_G - 1))

        ig_chunk(0)
        ig_chunk(1)

        # ---- alpha matmuls (fp8 weights; pre-activations 256x scaled) --
        for ko in range(KO_A):
            nc.tensor.matmul(pwi[:], lhsT=xt_t[:, KO_A + ko : KO_A + ko + 1],
                             rhs=wai_t[:, ko, :],
                             start=(ko == 0), stop=(ko == KO_A - 1))
        for ko in range(KO_A):
            nc.tensor.matmul(pal[:], lhsT=ct_t[:, ko, :],
                             rhs=wah_t[:, ko, :],
                             start=(ko == 0), stop=False)
        wi_t = singles.tile([1, HS], F32, tag="wi")
        nc.vector.tensor_add(out=wi_t[:], in0=pwi[:], in1=bab_t[:, 3 * HS : 4 * HS])
        nc.tensor.matmul(pal[:], lhsT=ones_b[0:1, 0:C], rhs=wi_t[:],
                         start=False, stop=True)

        # alpha chain on ACT/DVE (overlaps the PE's remaining ig chunks):
        # sigma via exp + fast reciprocal, then ew = exp(sigma), mg = cs * ew
        tmp_a = singles.tile([C, HS], F32, tag="tmp_a")
        nc.scalar.activation(out=tmp_a[:], in_=pal[:], func=EXP, scale=-1.0 / ASCALE)
        nc.vector.tensor_scalar_add(out=tmp_a[:], in0=tmp_a[:], scalar1=1.0)
        nc.vector.reciprocal_approx_fast(out=tmp_a[:], in_=tmp_a[:])
        nc.scalar.activation(out=emw[0:C, 0:HS], in_=tmp_a[:], func=EXP)
        nc.vector.tensor_mul(out=emw[0:C, HS : 2 * HS], in0=emw[0:C, HS : 2 * HS],
                             in1=emw[0:C, 0:HS])

        for ci in range(2, len(IG_SIZES)):
            ig_chunk(ci)

        # alpha-row reduction: [sum ew | sum mg] in one K=64 matmul
        nc.tensor.matmul(ps01[:], lhsT=ones_r[0:C, :], rhs=emw[0:C, :],
                         start=True, stop=True)

        # ---- i/g tail chain (ACT/DVE; overlaps the o stream) -----------
        # Every ACT exp in the kernel is done by now, so sigmoid-family ops
        # are safe (the direction exp->family never reloads a table).
        # ew64 = exp(sigma(pre_i)) via the quadratic in t = tanh(pre_i/2)/2:
        #   exp(sigma) = sqrt(e)*exp(t) ~= sqrt(e)*(1 + t + t^2/2), t in
        #   (-1/2, 1/2); max rel error ~1.4%, and c1's sensitivity to this
        #   softmax weight is ~1/65, so the contribution is ~2e-4.
        SQE = 1.6487212707001282
        th_t = singles.tile([1, HS], F32, tag="th")
        nc.scalar.activation(out=th_t[:], in_=pg_ig[:, 0:HS], func=TANH, scale=0.5 / ASCALE)
        # g row: direct tanh
        nc.scalar.activation(out=emw[C : C + 1, HS : 2 * HS],
                             in_=pg_ig[:, HS : 2 * HS], func=TANH,
                             scale=1.0 / ASCALE)
        p_t = singles.tile([1, HS], F32, tag="p")
        nc.vector.tensor_scalar(out=p_t[:], in0=th_t[:],
                                scalar1=SQE / 8.0, scalar2=SQE / 2.0,
                                op0=mybir.AluOpType.mult,
                                op1=mybir.AluOpType.add)
        nc.vector.tensor_mul(out=p_t[:], in0=p_t[:], in1=th_t[:])
        nc.vector.tensor_scalar_add(out=emw[C : C + 1, 0:HS], in0=p_t[:],
                                    scalar1=SQE)
        nc.vector.tensor_mul(out=emw[C : C + 1, HS : 2 * HS],
                             in0=emw[C : C + 1, HS : 2 * HS],
                             in1=emw[C : C + 1, 0:HS])

        # close the reduction with the gate row on DVE (no PE involved)
        s01_t = singles.tile([1, 2 * HS], F32, tag="s01")
        nc.vector.tensor_add(out=s01_t[:], in0=ps01[:], in1=emw[C : C + 1, :])

        # c1 = s1 / s0 (s0 = sum of 65 exp values in [1, e]; the ~18-bit
        # fast reciprocal is plenty), then tanh(c1); all of this overlaps
        # the o stream.  ht = tanh(c1)/2 feeds the final h1 product.
        r_t = singles.tile([1, HS], F32, tag="r")
        nc.vector.reciprocal_approx_fast(out=r_t[:], in_=s01_t[:, 0:HS])
        hc_t = singles.tile([1, 2 * HS], F32, tag="hc")
        nc.vector.tensor_mul(out=hc_t[:, 0:HS], in0=s01_t[:, HS : 2 * HS], in1=r_t[:])
        t4_t = singles.tile([1, HS], F32, tag="t4")
        nc.scalar.activation(out=t4_t[:], in_=hc_t[:, 0:HS], func=TANH)
        nc.vector.tensor_scalar_mul(out=t4_t[:], in0=t4_t[:], scalar1=0.5)
        nc.scalar.dma_start(out=hc[:, 0:HS], in_=hc_t[:, 0:HS])

        # ---- o gate column stream (last bytes of the kernel) -----------
        kk = 0
        for sz in O_SIZES:
            wo_t = wo_pool.tile([128, max(O_SIZES), HS], BF16, tag="wo")
            nc.sync.dma_start(out=wo_t[:, 0:sz, :], in_=wgo[:, kk : kk + sz, :])
            for km in range(sz):
                nc.tensor.matmul(pg_o[:], lhsT=xt_t[:, kk + km : kk + km + 1],
                                 rhs=wo_t[:, km, :],
                                 start=False, stop=(kk + km == KO_G - 1))
            kk += sz

        # Padding transfer nothing waits on: the last ~70 KB of a long
        # HWDGE stream drain at a trickle (~4 us); this makes those bytes
        # disposable pad instead of the o-gate weights the tail needs.
        pad_t = singles.tile([128, 8, 2 * HS], FP8, tag="pad")
        nc.sync.dma_start(out=pad_t[:], in_=wgig[:, 0:8, :])

        # ---- final o tail: h1 = sigma(pre_o) * tanh(c1) ----------------
        # sigma(x)*t = (1 + tanh(x/2)) * (t/2): stays on the resident tanh
        # table (a Sigmoid op would trigger a ~1.3 us table load here).
        so_t = singles.tile([1, HS], F32, tag="so")
        nc.scalar.activation(out=so_t[:], in_=pg_o[:], func=TANH, scale=0.5)
        nc.vector.tensor_scalar_add(out=so_t[:], in0=so_t[:], scalar1=1.0)
        nc.vector.tensor_mul(out=hc_t[:, HS : 2 * HS], in0=so_t[:], in1=t4_t[:])
        nc.sync.dma_start(out=hc[:, HS : 2 * HS], in_=hc_t[:, HS : 2 * HS])


def _shard_inputs(input_, c_input, h0, c0, weight_ih, weight_hh,
                  alpha_weight_ih, alpha_weight_hh, bias, alpha_bias):
    """Host-side scatter: column-shard the weights over the hidden dim.

    Weight matrices are cast to bf16 and pre-tiled to the [ki=128, ko, n]
    SBUF layout once for all cores; per-core shards are then cheap slices.
    """
    import ml_dtypes
    f32 = np.float32
    bf16 = ml_dtypes.bfloat16
    fp8 = ml_dtypes.float8_e4m3

    x_comb = np.concatenate([h0[0], input_[0]]).astype(f32)          # (4096,)
    xt = np.ascontiguousarray(x_comb.reshape(KO_G, 128).T).astype(bf16)
    # c_input.T tiled to [ki=128, ko=16, C]
    ct = np.ascontiguousarray(
        c_input.T.reshape(KO_A, 128, C).transpose(1, 0, 2)).astype(fp8)
    ones1 = np.ones((C, 1), f32)

    # gates weights: stack [W_hh; W_ih]; columns regrouped to [i | g] and
    # [o]; cast to bf16, tile to [128, 32, n].
    wg_full = np.concatenate([weight_hh, weight_ih], axis=0).astype(f32)
    wi_cols = wg_full[:, 0:H]
    wo_cols = wg_full[:, H : 2 * H]
    wgg_cols = wg_full[:, 2 * H : 3 * H]

    def ktile(a, dt):  # (4096, n) -> [128, 32, n]
        return np.ascontiguousarray(
            a.astype(dt).reshape(KO_G, 128, -1).transpose(1, 0, 2))

    wgi_t = ktile(wi_cols * ASCALE, fp8)
    wgg_t = ktile(wgg_cols * ASCALE, fp8)
    wgo_t = ktile(wo_cols, bf16)
    del wg_full, wi_cols, wo_cols, wgg_cols

    wai_t = np.ascontiguousarray(
        (alpha_weight_ih * ASCALE).astype(fp8).reshape(KO_A, 128, H).transpose(1, 0, 2))
    wah_t = np.ascontiguousarray(
        (alpha_weight_hh * ASCALE).astype(fp8).reshape(KO_A, 128, H).transpose(1, 0, 2))

    bias = np.asarray(bias, f32)
    alpha_bias = np.asarray(alpha_bias, f32)
    c_input = np.asarray(c_input, f32)

    in_maps = []
    for k in range(NCORES):
        cols = np.s_[k * HS : (k + 1) * HS]
        wgig = np.ascontiguousarray(
            np.concatenate([wgi_t[:, :, cols], wgg_t[:, :, cols]], axis=2))
        bab = np.concatenate(
            [bias[0 * H + k * HS : 0 * H + (k + 1) * HS] * ASCALE,
             bias[2 * H + k * HS : 2 * H + (k + 1) * HS] * ASCALE,
             bias[1 * H + k * HS : 1 * H + (k + 1) * HS],
             alpha_bias[cols] * ASCALE])[None, :].astype(f32)
        in_maps.append({
            "xt": xt,
            "ct": ct,
            "wai": np.ascontiguousarray(wai_t[:, :, cols]),
            "wah": np.ascontiguousarray(wah_t[:, :, cols]),
            "wgig": wgig,
            "wgo": np.ascontiguousarray(wgo_t[:, :, cols]),
            "bab": bab,
            "cs": np.ascontiguousarray(c_input[:, cols]),
            "ones1": ones1,
        })
    return in_maps


def _run(inputs, trace=False):
    global _nc_cache
    if _nc_cache is None:
        _nc_cache = _build_nc()
    nc = _nc_cache
    in_maps = _shard_inputs(**inputs)
    res = run_bass_kernel_spmd(nc, in_maps, core_ids=list(range(NCORES)), trace=trace)
    h1 = np.concatenate(
        [res.results[k]["hc"][:, HS : 2 * HS] for k in range(NCORES)], axis=1)
    c1 = np.concatenate(
        [res.results[k]["hc"][:, 0:HS] for k in range(NCORES)], axis=1)
    return (h1.astype(np.float32), c1.astype(np.float32)), res


def kernel(input_, c_input, h0, c0, weight_ih, weight_hh,
           alpha_weight_ih, alpha_weight_hh, bias, alpha_bias):
    inputs = dict(
        input_=np.asarray(input_, np.float32),
        c_input=np.asarray(c_input, np.float32),
        h0=np.asarray(h0, np.float32),
        c0=np.asarray(c0, np.float32),
        weight_ih=np.asarray(weight_ih, np.float32),
        weight_hh=np.asarray(weight_hh, np.float32),
        alpha_weight_ih=np.asarray(alpha_weight_ih, np.float32),
        alpha_weight_hh=np.asarray(alpha_weight_hh, np.float32),
        bias=np.asarray(bias, np.float32),
        alpha_bias=np.asarray(alpha_bias, np.float32),
    )
    out, _ = _run(inputs)
    return out
